# revision 1
# baseline (speedup 1.0000x reference)
"""Trainium2 Bass kernel for the pairwise-similarity histogram loss.

Reference computation:
  sim = x @ x.T  (rows L2-normalized), upper-tri pairs (i<j)
  soft (triangular) binning of similarities into 51 bins, separately for
  label-equal (pos) and label-unequal (neg) pairs; loss = sum(hist_neg * cumsum(hist_pos)).

Device algorithm (8 NeuronCores, SPMD, data-parallel over sim rows):
  Host sorts rows by label and hands each core a column-rotated copy of
  x_sorted.T so every core's own 128 rows sit at rotated columns 0..127 —
  all cores run an identical program.

  Per core:
    - PE: sim_shard = xT[:,0:128].T @ xT   -> PSUM [128, 1024]
    - masks from iota/label compares; s' = triu_mask * (1 + sim) in [0, 2]
    - histogramming uses the identity
        F[k] = sum_p clip((s'_p - k*bw)/bw, 0, 1) = (R[k] - R[k+1]) / bw,
        R[k] = sum_p relu(s'_p - k*bw)
      each R[k] is ONE fused instruction (relu + per-partition accumulate)
      on DVE (tensor_scalar sub/max + accum_out) or ACT (activation Relu +
      accum_out), split across engines.
    - pos pairs: after label-sorting they all live in a 64-wide diagonal
      band; the band is extracted via a skewed DRAM bounce and binned on a
      tiny [128, 63] tile.  neg = triu - pos.
  Host: f64 combine of per-partition partial sums -> final scalar loss.
"""

import numpy as np

NBINS = 51
BW = 2.0 / (NBINS - 1)
BS, D = 1024, 128
N_CORES = 8
SH = BS // N_CORES  # 128 rows per core

# ---------------- configuration ----------------
USE_BAND = False          # pos histogram via diagonal band (else dense pos passes)
KLO, KHI = 13, 38          # R[k] computed on device for k in [KLO, KHI]; outside: closed form
PASS_DT = "float16"       # dtype of the masked s' tiles the bin passes read
BANDW = 64                # band width (covers within-class pair distance <= 63)

_CACHE = {}


def _build_program():
    import concourse.bass as bass
    import concourse.bacc as bacc
    import concourse.tile as tile
    import concourse.mybir as mybir
    from concourse.ap import AP

    F32 = mybir.dt.float32
    PDT = getattr(mybir.dt, PASS_DT)
    Alu = mybir.AluOpType
    Act = mybir.ActivationFunctionType

    ks = list(range(KLO, KHI + 1))  # R[k] passes on device

    # pass plan: (family, k) -> engine + column index
    passes = []
    fams = ["tri", "pos"] if USE_BAND else ["pos", "neg"]
    for fam in fams:
        for k in ks:
            passes.append((fam, k))
    # weighted greedy assignment across DVE/ACT/GPSIMD by modeled pass cost
    est = {"D": 2300.0, "A": 1250.0, "G": 2200.0}  # head start = prep busy
    cost = {"D": 1147.0, "A": 1253.0, "G": 1e12}
    plan = {}
    counts = {"D": 0, "A": 0, "G": 0}
    for fam, k in passes:
        eng = min(est, key=lambda e: est[e] + cost[e])
        est[eng] += cost[eng]
        plan[(fam, k)] = (eng, None)
    # column indexing per engine
    for key in plan:
        eng, _ = plan[key]
        plan[key] = (eng, counts[eng])
        counts[eng] += 1
    nD, nA, nG = counts["D"], counts["A"], counts["G"]
    NCNT = 4  # cntpos, cntneg, Spos, Sneg
    NOUT = nD + nA + nG + NCNT

    nc = bacc.Bacc("TRN2", target_bir_lowering=False, debug=False,
                   num_devices=N_CORES)

    xTrL = nc.dram_tensor("xTrL", [D, 512], F32, kind="ExternalInput")
    xTrR = nc.dram_tensor("xTrR", [D, 512], F32, kind="ExternalInput")
    collab = nc.dram_tensor("collab", [1, BS], F32, kind="ExternalInput")
    collab_bf = nc.dram_tensor("collab_bf", [1, BS], mybir.dt.bfloat16,
                               kind="ExternalInput")
    rowlab = nc.dram_tensor("rowlab", [SH, 1], F32, kind="ExternalInput")
    wrapcut = nc.dram_tensor("wrapcut", [SH, 1], F32, kind="ExternalInput")
    nk = len(ks)
    cvec = nc.dram_tensor("cvec", [SH, nk], F32, kind="ExternalInput")
    acc_out = nc.dram_tensor("acc", [SH, NOUT], F32, kind="ExternalOutput")

    with tile.TileContext(nc) as tc:
        with tc.tile_pool(name="main", bufs=1) as pool, \
             tc.tile_pool(name="psum", bufs=1, space="PSUM") as psum:
            xL = pool.tile([D, 512], F32)
            xR = pool.tile([D, 512], F32)
            nc.sync.dma_start(xL[:], xTrL[:])
            nc.sync.dma_start(xR[:], xTrR[:])

            rowlab_sb = pool.tile([SH, 1], F32)
            nc.sync.dma_start(rowlab_sb[:], rowlab[:])
            wrapcut_sb = pool.tile([SH, 1], F32)
            nc.sync.dma_start(wrapcut_sb[:], wrapcut[:])
            cvec_sb = pool.tile([SH, nk], F32)
            nc.sync.dma_start(cvec_sb[:], cvec[:])
            collab_bf_sb = pool.tile([1, BS], mybir.dt.bfloat16)
            nc.sync.dma_start(collab_bf_sb[:], collab_bf[:])

            ones_bf = pool.tile([1, D], mybir.dt.bfloat16)
            nc.vector.memset(ones_bf[:], 1.0)

            # iota along free dim (0..1023), and local row index (0..127)
            iotaT = pool.tile([SH, BS], F32)
            nc.gpsimd.iota(iotaT[:], pattern=[[1, BS]], base=0,
                           channel_multiplier=0,
                           allow_small_or_imprecise_dtypes=True)
            rowloc = pool.tile([SH, 1], F32)
            nc.gpsimd.iota(rowloc[:], pattern=[[0, 1]], base=0,
                           channel_multiplier=1,
                           allow_small_or_imprecise_dtypes=True)

            # sim = xT[:, 0:128].T @ xT  -> PSUM
            simP = psum.tile([SH, BS], F32)
            nc.tensor.matmul(simP[:, 0:512], xL[:, 0:D], xL[:])
            nc.tensor.matmul(simP[:, 512:BS], xL[:, 0:D], xR[:])

            # label broadcast via K=1 bf16 matmul (labels 0..31 exact in bf16)
            labmatP = psum.tile([SH, BS], F32)
            nc.tensor.matmul(labmatP[:, 0:512], ones_bf[:], collab_bf_sb[:, 0:512])
            nc.tensor.matmul(labmatP[:, 512:BS], ones_bf[:], collab_bf_sb[:, 512:BS])

            # triu mask (in rotated coords): (t > r) & (t < wrapcut)
            gtmask = pool.tile([SH, BS], F32)
            nc.gpsimd.tensor_scalar(gtmask[:], iotaT[:], rowloc[:], None,
                                    op0=Alu.is_gt)
            trimask = pool.tile([SH, BS], F32)
            nc.vector.scalar_tensor_tensor(trimask[:], iotaT[:], wrapcut_sb[:],
                                           gtmask[:], op0=Alu.is_lt, op1=Alu.mult)

            # s' = 1 + sim (ACT, evacuates PSUM)
            splus = pool.tile([SH, BS], F32)
            nc.scalar.activation(splus[:], simP[:], Act.Identity, bias=1.0)

            cnts = pool.tile([SH, NCNT], F32)

            # pos/neg masks and masked s' tensors
            mpos = pool.tile([SH, BS], F32)
            nc.vector.scalar_tensor_tensor(mpos[:], labmatP[:], rowlab_sb[:],
                                           trimask[:], op0=Alu.is_equal,
                                           op1=Alu.mult,
                                           accum_out=cnts[:, 0:1])
            mneg = pool.tile([SH, BS], F32)
            nc.vector.scalar_tensor_tensor(mneg[:], trimask[:], 1.0, mpos[:],
                                           op0=Alu.mult, op1=Alu.subtract,
                                           accum_out=cnts[:, 1:2])
            spos = pool.tile([SH, BS], PDT)
            nc.vector.scalar_tensor_tensor(spos[:], mpos[:], 1.0, splus[:],
                                           op0=Alu.mult, op1=Alu.mult)
            sneg = pool.tile([SH, BS], PDT)
            nc.vector.scalar_tensor_tensor(sneg[:], mneg[:], 1.0, splus[:],
                                           op0=Alu.mult, op1=Alu.mult)
            src = {"pos": spos, "neg": sneg}

            # sums of masked s' (for closed-form low bins)
            trashD = pool.tile([SH, BS], PDT)
            trashA = pool.tile([SH, BS], PDT)
            nc.vector.tensor_scalar(trashD[:], spos[:], 1.0, 0.0, op0=Alu.mult,
                                    op1=Alu.add, accum_out=cnts[:, 2:3])
            nc.vector.tensor_scalar(trashA[:], sneg[:], 1.0, 0.0, op0=Alu.mult,
                                    op1=Alu.add, accum_out=cnts[:, 3:4])

            zeros = pool.tile([SH, BS], PDT)
            nc.vector.memset(zeros[:], 0.0)
            accD = pool.tile([SH, max(nD, 1)], F32)
            accG = pool.tile([SH, max(nG, 1)], F32)
            trashG = pool.tile([SH, BS], PDT)
            accA = pool.tile([SH, max(nA, 1)], F32)

            for fam, k in passes:
                eng, j = plan[(fam, k)]
                c = float(np.float32(k * BW))
                s_t = src[fam]
                if eng == "D":
                    nc.vector.scalar_tensor_tensor(trashD[:], s_t[:], c,
                                                   zeros[:], op0=Alu.subtract,
                                                   op1=Alu.max,
                                                   accum_out=accD[:, j:j + 1])
                elif eng == "G":
                    nc.gpsimd.scalar_tensor_tensor(trashG[:], s_t[:], c,
                                                   zeros[:], op0=Alu.subtract,
                                                   op1=Alu.max,
                                                   accum_out=accG[:, j:j + 1])
                else:
                    jc = k - KLO
                    nc.scalar.activation(trashA[:], s_t[:], Act.Relu,
                                         bias=cvec_sb[:, jc:jc + 1], scale=1.0,
                                         accum_out=accA[:, j:j + 1])

            nc.sync.dma_start(acc_out[:, 0:nD], accD[:])
            nc.sync.dma_start(acc_out[:, nD:nD + nA], accA[:])
            if nG:
                nc.sync.dma_start(acc_out[:, nD + nA:nD + nA + nG], accG[:])
            nc.sync.dma_start(acc_out[:, nD + nA + nG:NOUT], cnts[:])

    nc.compile()
    return nc, plan, (nD, nA, nG, NOUT)


def _get_program():
    key = (USE_BAND, KLO, KHI, PASS_DT)
    if key not in _CACHE:
        _CACHE[key] = _build_program()
    return _CACHE[key]


def _host_prep(x, labels):
    x = np.ascontiguousarray(np.asarray(x, dtype=np.float32))
    labels = np.asarray(labels).astype(np.int64)
    perm = np.argsort(labels, kind="stable")
    xs = x[perm]
    labs = labels[perm].astype(np.float32)
    xT = np.ascontiguousarray(xs.T)  # [128, 1024]
    import ml_dtypes
    in_maps = []
    for c in range(N_CORES):
        r = SH * c
        xTr = np.roll(xT, -r, axis=1)
        collab_c = np.ascontiguousarray(np.roll(labs, -r)[None, :])
        rowlab_c = np.ascontiguousarray(collab_c[0, :SH, None])
        wrapcut_c = np.full((SH, 1), float(BS - r), np.float32)
        ks_arr = np.arange(KLO, KHI + 1, dtype=np.float32)
        cvec_c = np.tile(-(ks_arr * np.float32(BW))[None, :], (SH, 1)).astype(np.float32)
        in_maps.append({
            "cvec": cvec_c,
            "xTrL": np.ascontiguousarray(xTr[:, 0:512]),
            "xTrR": np.ascontiguousarray(xTr[:, 512:]),
            "collab": collab_c,
            "collab_bf": collab_c.astype(ml_dtypes.bfloat16),
            "rowlab": rowlab_c,
            "wrapcut": wrapcut_c,
        })
    return in_maps, labels


def _combine(results, plan, meta, labels):
    nD, nA, nG, NOUT = meta
    tot = np.zeros((NOUT,), np.float64)
    gmax = np.full((NOUT,), -np.inf)
    for res in results:
        a = res["acc"].astype(np.float64)
        tot += a.sum(axis=0)
        gmax = np.maximum(gmax, a.max(axis=0))

    def col(eng, j):
        return {"D": 0, "A": nD, "G": nD + nA}[eng] + j

    base = nD + nA + nG
    cntpos = tot[base + 0]
    cntneg = tot[base + 1]
    Spos = tot[base + 2]
    Sneg = tot[base + 3]
    npairs = BS * (BS - 1) // 2
    assert abs(cntpos + cntneg - npairs) < 0.5, (cntpos, cntneg)
    # range guards (zero device cost): R[KLO] must match the closed form
    # (no real value below KLO*BW) and R[KHI] must be ~0 (none above).
    def Rdev(fam, k):
        eng, j = plan[(fam, k)]
        return tot[col(eng, j)]
    ok = True
    for fam, Sm, Nm in (("pos", Spos, cntpos), ("neg", Sneg, cntneg)):
        ok &= abs(Rdev(fam, KLO) - (Sm - Nm * KLO * BW)) < 0.5
        ok &= Rdev(fam, KHI) < 0.5
    if not ok:
        return None  # out-of-range: caller falls back to exact host path

    def R_of(fam, Sm, Nm):
        R = np.zeros((NBINS + 1,), np.float64)  # k = 0..51
        for k in range(NBINS + 1):
            if k < KLO:
                R[k] = Sm - Nm * (k * BW)
            elif k > KHI:
                R[k] = 0.0
            else:
                eng, j = plan[(fam, k)]
                R[k] = tot[col(eng, j)]
        return R

    Rpos = R_of("pos", Spos, cntpos)
    Rneg = R_of("neg", Sneg, cntneg)
    Fpos = (Rpos[:-1] - Rpos[1:]) / BW          # k = 0..50
    Fneg = (Rneg[:-1] - Rneg[1:]) / BW
    Fneg_m1 = cntneg
    histneg = np.empty((NBINS,), np.float64)
    histneg[0] = (Fneg_m1 - Fneg[0]) / cntneg
    histneg[1:] = (Fneg[:-1] - Fneg[1:]) / cntneg
    cdfpos = 1.0 - Fpos / cntpos
    loss = float(np.sum(histneg * cdfpos))
    return np.float32(loss)


def _host_exact(x, labels):
    # exact fallback, only used if the data violates the compiled bin range
    x = np.asarray(x, np.float64)
    labels = np.asarray(labels)
    sim = x @ x.T
    iu, ju = np.triu_indices(x.shape[0], k=1)
    s = sim[iu, ju]
    pos = labels[iu] == labels[ju]
    b = np.floor((s + 1.0) / BW).astype(np.int64)
    v = b * BW - 1.0
    w_lo = (v + BW - s) / BW
    w_hi = (s - v) / BW
    b_hi = np.clip(b + 1, 0, NBINS - 1)

    def hist(m):
        h = np.zeros(NBINS)
        np.add.at(h, b[m], w_lo[m])
        np.add.at(h, b_hi[m], w_hi[m])
        return h / m.sum()

    hp, hn = hist(pos), hist(~pos)
    return np.float32(np.sum(hn * np.cumsum(hp)))


def _run(x, labels, trace=False, trace_cores=None):
    from concourse.bass_utils import run_bass_kernel_spmd
    nc, plan, meta = _get_program()
    in_maps, labels = _host_prep(x, labels)
    out = run_bass_kernel_spmd(nc, in_maps, list(range(N_CORES)),
                               trace=trace, trace_cores=trace_cores)
    loss = _combine(out.results, plan, meta, labels)
    if loss is None:
        loss = _host_exact(x, labels)
    return loss, out


def kernel(x, labels):
    loss, _ = _run(x, labels)
    return loss



# revision 5
# speedup vs baseline: 2.9932x; 2.9932x over previous
"""Trainium2 Bass kernel for the pairwise-similarity histogram loss.

Reference computation:
  sim = x @ x.T  (rows L2-normalized), upper-tri pairs (i<j)
  soft (triangular) binning of similarities into 51 bins, separately for
  label-equal (pos) and label-unequal (neg) pairs; loss = sum(hist_neg * cumsum(hist_pos)).

Device algorithm (8 NeuronCores, SPMD, data-parallel over pairs):
  Host sorts rows by label. Pairs are enumerated by cyclic offset: pair
  {a, b} with d = b - a is covered by row a at offset d when d <= 512,
  else by row b at offset 1024 - d.  Core c (rows r..r+127, r = 128c)
  therefore needs sim[g, (g+o) mod 1024] for o in [1, 512] only - a
  diagonal band inside a [128, 640] tile computed as ONE f16 matmul of
  xT[:, r:r+128].T @ xT[:, r+1:r+641] (cyclic slice, host-prepared).

  The band (valid run of <= 512 consecutive cols per row) folds onto a
  dense [128, 512] tile by adding cols [512:640) onto [0:128).  After
  label sorting all pos (same-label) pairs live at offsets 1..63; a
  host-built mask extracts them from cols [0:192) and a 3-way fold
  packs them into [128, 64].  neg = all - pos is formed on the host.

  Histogramming uses R[k] = sum_p relu(s'_p - k*bw), s' = 1 + sim,
  F[k] = (R[k] - R[k+1]) / bw, one fused instruction per R[k]:
  tensor_scalar (DVE, 4x f16 perf mode), activation Relu (ACT), or
  gpsimd tensor_scalar (Pool), with per-partition accum_out.  Outside
  the compiled k-range R is closed-form; device guard passes verify the
  range (else exact host fallback).  Counts come from labels (host).

  Host: f64 combine of per-partition partials -> final scalar loss.
"""

import numpy as np

NBINS = 51
BW = 2.0 / (NBINS - 1)
BS, D = 1024, 128
N_CORES = 8
SH = BS // N_CORES      # 128 rows per core
TILE_W = 640            # band tile width
WALL = 512              # folded all-pairs tile width
WPOS = 64               # folded pos tile width
POSW = 192              # pos extraction strip (covers offsets 1..63 for all rows)
XW = 641                # x slice width: lhsT cols [0:128], rhs cols [1:641]
BANDO = 63              # max same-label pair offset supported

KLO_A, KHI_A = 15, 36   # device R[k] range, all pairs (R[KHI_A] ~ 0 is the high guard)
KLO_P, KHI_P = 16, 35   # device R[k] range, pos pairs

_CACHE = {}


def _plan_passes():
    """Assign (family, k) histogram passes to engines by balancing modeled cost."""
    passes_all = [("all", "r", k) for k in range(KLO_A, KHI_A + 1)] + \
                 [("all", "min", KLO_A)]
    passes_pos = [("pos", "r", k) for k in range(KLO_P, KHI_P + 1)] + \
                 [("pos", "min", KLO_P)]
    n_all, n_pos = len(passes_all), len(passes_pos)
    # GPSIMD cannot run accum-bearing dual-op passes (walrus engine check),
    # so histogram passes go to DVE (4x f16 tensor_scalar) and ACT only.
    cost = {"D": {"all": 194.0, "pos": 77.0},
            "A": {"all": 799.0, "pos": 425.0}}
    start = {"D": 1000.0, "A": 500.0}
    best = None
    for aA in range(0, 12):
        for pA in range(0, 12):
            aD = n_all - aA
            pD = n_pos - pA
            if aD < 0 or pD < 0:
                continue
            f = {"D": start["D"] + aD * cost["D"]["all"] + pD * cost["D"]["pos"],
                 "A": start["A"] + aA * cost["A"]["all"] + pA * cost["A"]["pos"]}
            m = max(f.values())
            if best is None or m < best[0]:
                best = (m, aA, pA)
    _, aA, pA = best
    aG = pG = 0
    aD = n_all - aA - aG
    pD = n_pos - pA - pG
    # ACT cannot express the min-guard -> give ACT "r" passes only (from the
    # middle of the k range); guards go to DVE/Pool.
    plan = {}  # (fam, kind, k) -> engine
    def split(passes, nD, nA, nG):
        rs = [p for p in passes if p[1] == "r"]
        gmin = [p for p in passes if p[1] == "min"]
        # ACT takes nA from the middle of the r-range
        mid = len(rs) // 2
        a_lo = mid - nA // 2
        act = rs[a_lo:a_lo + nA]
        rest = [p for p in rs if p not in act] + gmin
        pool_p = rest[-nG:] if nG else []
        dve = [p for p in rest if p not in pool_p]
        assert len(dve) == nD
        for p in act:
            plan[p] = "A"
        for p in pool_p:
            plan[p] = "G"
        for p in dve:
            plan[p] = "D"
    split(passes_all, aD, aA, aG)
    split(passes_pos, pD, pA, pG)
    return passes_all, passes_pos, plan


def _build_program():
    import concourse.bass as bass
    import concourse.bacc as bacc
    import concourse.tile as tile
    import concourse.mybir as mybir

    F32 = mybir.dt.float32
    F16 = mybir.dt.float16
    Alu = mybir.AluOpType
    Act = mybir.ActivationFunctionType

    passes_all, passes_pos, plan = _plan_passes()
    # emission order per engine: pos passes first (their tile is ready earlier)
    order = [p for p in passes_pos if plan[p] == "D"] + \
            [p for p in passes_all if plan[p] == "D"]
    orderA = [p for p in passes_pos if plan[p] == "A"] + \
             [p for p in passes_all if plan[p] == "A"]
    orderG = [p for p in passes_pos if plan[p] == "G"] + \
             [p for p in passes_all if plan[p] == "G"]
    nD, nA, nG = len(order), len(orderA), len(orderG)
    NOUT = nD + nA + nG
    # acc column map
    colmap = {}
    for j, p in enumerate(order):
        colmap[p] = j
    for j, p in enumerate(orderA):
        colmap[p] = nD + j
    for j, p in enumerate(orderG):
        colmap[p] = nD + nA + j
    act_bias = [(-(k * BW) if kind == "r" else None) for (_, kind, k) in orderA]
    assert all(b is not None for b in act_bias)

    nc = bacc.Bacc("TRN2", target_bir_lowering=False, debug=False,
                   num_devices=N_CORES)

    x_d = nc.dram_tensor("xs", [D, XW], F16, kind="ExternalInput")
    mpack_d = nc.dram_tensor("mpack", [SH, 512], F16, kind="ExternalInput")
    cvec_d = nc.dram_tensor("cvec", [SH, max(nA, 1)], F32, kind="ExternalInput")
    acc_d = nc.dram_tensor("acc", [SH, NOUT], F32, kind="ExternalOutput")

    with tile.TileContext(nc) as tc:
        with tc.tile_pool(name="main", bufs=1) as pool, \
             tc.tile_pool(name="psum", bufs=1, space="PSUM") as psum:
            xs = pool.tile([D, XW], F16)
            nc.sync.dma_start(xs[:], x_d[:])
            mpack = pool.tile([SH, 512], F16)
            nc.sync.dma_start(mpack[:], mpack_d[:])
            cvec_sb = pool.tile([SH, max(nA, 1)], F32)
            nc.sync.dma_start(cvec_sb[:], cvec_d[:])

            simP = psum.tile([SH, TILE_W], F32)
            nc.tensor.matmul(simP[:, 0:POSW], xs[:, 0:D], xs[:, 1:1 + POSW])
            nc.tensor.matmul(simP[:, POSW:512], xs[:, 0:D], xs[:, 1 + POSW:513])
            nc.tensor.matmul(simP[:, 512:640], xs[:, 0:D], xs[:, 513:641])

            # pos chain: fused evac+mask from PSUM, then 3 -> 1 fold
            sposR = pool.tile([SH, POSW], F16)
            nc.vector.scalar_tensor_tensor(sposR[:], simP[:, 0:POSW], 1.0,
                                           mpack[:, 320:512],
                                           op0=Alu.add, op1=Alu.mult)
            pfold = pool.tile([SH, WPOS], F16)
            nc.vector.tensor_tensor(pfold[:], sposR[:, 0:64], sposR[:, 64:128],
                                    op=Alu.add)
            ppos = pool.tile([SH, WPOS], F16)
            nc.vector.tensor_tensor(ppos[:], pfold[:], sposR[:, 128:192],
                                    op=Alu.add)

            # all chain: strip evac+mask on DVE, middle on ACT, 2 -> 1 fold
            sall = pool.tile([SH, TILE_W], F16)
            nc.vector.scalar_tensor_tensor(sall[:, 0:128], simP[:, 0:128], 1.0,
                                           mpack[:, 0:128],
                                           op0=Alu.add, op1=Alu.mult)
            nc.scalar.activation(sall[:, 128:448], simP[:, 128:448],
                                 Act.Identity, bias=1.0)
            nc.vector.scalar_tensor_tensor(sall[:, 448:640], simP[:, 448:640],
                                           1.0, mpack[:, 128:320],
                                           op0=Alu.add, op1=Alu.mult)
            nc.vector.tensor_tensor(sall[:, 0:128], sall[:, 0:128],
                                    sall[:, 512:640], op=Alu.add)

            acc = pool.tile([SH, NOUT], F32)
            trashD = pool.tile([SH, WALL], F16)
            trashA = pool.tile([SH, WALL], F16)
            trashG = pool.tile([SH, WALL], F16)
            zerosG = pool.tile([SH, WALL], F16)
            if nG:
                nc.gpsimd.memset(zerosG[:], 0.0)

            def emit(eng, p):
                fam, kind, k = p
                j = colmap[p]
                src = sall[:, 0:WALL] if fam == "all" else ppos[:]
                w = WALL if fam == "all" else WPOS
                c = float(np.float32(k * BW))
                a = acc[:, j:j + 1]
                if eng == "A":
                    jc = orderA.index(p)
                    nc.scalar.activation(trashA[:, 0:w], src, Act.Relu,
                                         bias=cvec_sb[:, jc:jc + 1], scale=1.0,
                                         accum_out=a)
                elif eng == "D":
                    opi = Alu.subtract if kind == "r" else Alu.min
                    nc.vector.tensor_scalar(trashD[:, 0:w], src, c, 0.0,
                                            op0=opi, op1=Alu.max, accum_out=a)
                else:
                    opi = Alu.subtract if kind == "r" else Alu.min
                    nc.gpsimd.scalar_tensor_tensor(trashG[:, 0:w], src, c,
                                                   zerosG[:, 0:w], op0=opi,
                                                   op1=Alu.max, accum_out=a)

            # interleave source emission across engines, pos phase first
            queues = [("D", order), ("A", orderA), ("G", orderG)]
            idx = {e: 0 for e, _ in queues}
            remaining = nD + nA + nG
            while remaining:
                for e, q in queues:
                    if idx[e] < len(q):
                        emit(e, q[idx[e]])
                        idx[e] += 1
                        remaining -= 1

            nc.sync.dma_start(acc_d[:], acc[:])

    nc.compile()
    meta = (passes_all, passes_pos, plan, colmap, (nD, nA, nG, NOUT), act_bias)
    return nc, meta


def _get_program():
    key = (KLO_A, KHI_A, KLO_P, KHI_P)
    if key not in _CACHE:
        _CACHE[key] = _build_program()
    return _CACHE[key]


def _host_prep(x, labels, meta):
    passes_all, passes_pos, plan, colmap, (nD, nA, nG, NOUT), act_bias = meta
    x = np.ascontiguousarray(np.asarray(x, dtype=np.float32))
    labels = np.asarray(labels).astype(np.int64)
    perm = np.argsort(labels, kind="stable")
    ls = labels[perm]
    xT = np.ascontiguousarray(x[perm].T).astype(np.float16)  # [128, 1024]

    sizes = np.bincount(ls, minlength=1)
    band_ok = sizes.max() <= BANDO + 1

    il = np.arange(SH)[:, None]
    cvec_row = np.array(act_bias, np.float32)[None, :] if nA else \
        np.zeros((1, 1), np.float32)
    in_maps = []
    for c in range(N_CORES):
        r = SH * c
        cols = (np.arange(XW) + r) % BS
        x_c = np.ascontiguousarray(xT[:, cols])
        ex = 1 if r + SH <= BS // 2 else 0
        tL = np.arange(128)[None, :]
        mL = (tL >= il).astype(np.float16)
        tR = np.arange(448, 640)[None, :]
        mR = (tR <= il + 510 + ex).astype(np.float16)
        tP = np.arange(POSW)[None, :]
        g = r + il
        pairlab = ls[(g + tP + 1) % BS]
        mpos = ((tP >= il) & (tP <= il + BANDO - 1)
                & (pairlab == ls[g])).astype(np.float16)
        mpack = np.ascontiguousarray(
            np.concatenate([mL, mR, mpos], axis=1))
        in_maps.append({
            "xs": x_c,
            "mpack": mpack,
            "cvec": np.ascontiguousarray(np.broadcast_to(
                cvec_row, (SH, cvec_row.shape[1]))).astype(np.float32),
        })
    n_pos = int((sizes * (sizes - 1) // 2).sum())
    n_all = BS * (BS - 1) // 2
    return in_maps, band_ok, n_pos, n_all


def _combine(results, meta, band_ok, n_pos, n_all):
    passes_all, passes_pos, plan, colmap, (nD, nA, nG, NOUT), act_bias = meta
    if not band_ok:
        return None
    tot = np.zeros((NOUT,), np.float64)
    for res in results:
        tot += res["acc"].astype(np.float64).sum(axis=0)

    def val(fam, kind, k):
        return tot[colmap[(fam, kind, k)]]

    n_neg = n_all - n_pos

    # range guards: high R ~ 0 and min-clamp sum matches all-in-range value
    def guard(fam, klo, khi, n):
        if val(fam, "r", khi) > 1.0:
            return False
        c_lo = klo * BW
        m = val(fam, "min", klo)
        cands = [n * c_lo, n * float(np.float16(c_lo))]
        return min(abs(m - cd) for cd in cands) < 64.0

    if not (guard("all", KLO_A, KHI_A, n_all)
            and guard("pos", KLO_P, KHI_P, n_pos)):
        return None

    def F_of(fam, klo, khi, n):
        F = np.zeros((NBINS,), np.float64)
        F[:klo] = n
        R = np.array([val(fam, "r", k) for k in range(klo, khi + 1)] + [0.0])
        F[klo:khi] = (R[:-1] - R[1:])[: khi - klo] / BW
        # k >= khi: F stays 0  (R[khi] ~ 0 by guard)
        return F

    F_all = F_of("all", KLO_A, KHI_A, n_all)
    F_pos = F_of("pos", KLO_P, KHI_P, n_pos)
    F_neg = F_all - F_pos

    hist_neg = np.empty((NBINS,), np.float64)
    hist_neg[0] = (n_neg - F_neg[0]) / n_neg
    hist_neg[1:] = (F_neg[:-1] - F_neg[1:]) / n_neg
    cdf_pos = 1.0 - F_pos / n_pos
    return np.float32(np.sum(hist_neg * cdf_pos))


def _host_exact(x, labels):
    # exact fallback, only used if the data violates compiled assumptions
    x = np.asarray(x, np.float64)
    labels = np.asarray(labels)
    sim = x @ x.T
    iu, ju = np.triu_indices(x.shape[0], k=1)
    s = sim[iu, ju]
    pos = labels[iu] == labels[ju]
    b = np.floor((s + 1.0) / BW).astype(np.int64)
    v = b * BW - 1.0
    w_lo = (v + BW - s) / BW
    w_hi = (s - v) / BW
    b_hi = np.clip(b + 1, 0, NBINS - 1)

    def hist(m):
        h = np.zeros(NBINS)
        np.add.at(h, b[m], w_lo[m])
        np.add.at(h, b_hi[m], w_hi[m])
        return h / m.sum()

    hp, hn = hist(pos), hist(~pos)
    return np.float32(np.sum(hn * np.cumsum(hp)))


def _run(x, labels, trace=False, trace_cores=None):
    from concourse.bass_utils import run_bass_kernel_spmd
    nc, meta = _get_program()
    in_maps, band_ok, n_pos, n_all = _host_prep(x, labels, meta)
    out = run_bass_kernel_spmd(nc, in_maps, list(range(N_CORES)),
                               trace=trace, trace_cores=trace_cores)
    loss = _combine(out.results, meta, band_ok, n_pos, n_all)
    if loss is None:
        loss = _host_exact(x, labels)
    return loss, out


def kernel(x, labels):
    loss, _ = _run(x, labels)
    return loss
